# revision 19
# baseline (speedup 1.0000x reference)
"""CacheAwareMHA TRN2 kernel: 8-core head-sharded attention (fp16 PE path).

Strategy (v2):
  - Shard heads (16) across 8 cores: 2 heads/core. Each core holds W_q/W_o
    column/row shards and its heads' K/V slices; partial outputs summed on host.
  - ALL matmuls in float16 (1 cycle/col on PE vs ~3 for fp32); PSUM stays fp32.
    Tolerance is 2e-2; fp16 keeps rel err ~1e-3.
  - S^T layout [m partitions, t free] so softmax weights feed P@V directly.
  - Queries sorted by position on host -> causal mask is a per-(m-tile)
    t-prefix; windows are exact (aligned down to 16 cols for 32B ISA starts);
    the ~16-48 col partial band is masked with one scalar_tensor_tensor.
  - exp without max-subtraction (logits ~N(0,1); fp16 P max ~e^7 ok, row sums
    ~5e3 < fp16 max 65504).
  - Row sums on the DVE (in-place fp16 accumulate per m-tile, 4x mode) --
    frees ~20us of PE ones-matmul streaming.
  - Loads split across 4 DGE rings (vector: tables; sync+scalar: wq/x
    fine-grained; gpsimd: k/v/wo bulk) so Qproj starts at ~4us.
  - Output projection emitted fp16; partials summed on host.
"""
import sys
import math

import numpy as np

for _p in ("/opt/trn_rl_repo", "/opt/pypackages"):
    if _p not in sys.path:
        sys.path.append(_p)

T, D, H, DK, M = 1024, 2048, 16, 128, 4096
NCORES = 8
HLOC = H // NCORES  # heads per core
KO = D // 128       # 16 contraction tiles for projections
MT = M // 128       # 32 m-tiles
ROPE_BASE = 10000.0
SCALE = 1.0 / math.sqrt(DK)

_PROGRAM_CACHE = {}


def _host_rope_k(k, pos):
    """Apply RoPE to cached keys on host (fp64 tables). k: [M, h, DK]."""
    inv = 1.0 / (ROPE_BASE ** (np.arange(0, DK, 2, dtype=np.float64) / DK))
    th = pos[:, None].astype(np.float64) * inv[None, :]
    cos = np.concatenate([np.cos(th), np.cos(th)], -1)[:, None, :]
    sin = np.concatenate([np.sin(th), np.sin(th)], -1)[:, None, :]
    t1, t2 = k[..., :64], k[..., 64:]
    rot = np.concatenate([-t2, t1], -1)
    return (k.astype(np.float64) * cos + rot.astype(np.float64) * sin).astype(np.float32)


def _host_q_tables(pos_sorted):
    """cos / sign-baked sin tables in Q^T layout [DK, T] (fp32)."""
    inv = 1.0 / (ROPE_BASE ** (np.arange(0, DK, 2, dtype=np.float64) / DK))
    th = pos_sorted[None, :].astype(np.float64) * inv[:, None]      # [64, T]
    cos = np.cos(th)
    sin = np.sin(th)
    cosT = np.concatenate([cos, cos], 0).astype(np.float32)          # [128, T]
    sinT = np.concatenate([-sin, sin], 0).astype(np.float32)         # sign baked
    return cosT, sinT


def _windows(a_list, b_list):
    """Per m-tile: (chunk_lo[2], lo, stt_hi).

    lo = exact causal window start aligned down to 16 (32B fp16 matmul
    operand starts; PSUM fp32 outs need 8 -> 16 covers both). chunk_lo[c]
    is the start col for S/PV matmuls in 512-col PSUM-bank chunk c
    (None = chunk fully masked)."""
    out = []
    for i in range(MT):
        lo = a_list[i] & ~15
        chunk_lo = []
        for c in range(2):
            clo = max(512 * c, lo)
            chunk_lo.append(clo if clo < 512 * (c + 1) else None)
        out.append((chunk_lo, lo, b_list[i]))
    return out


def _build_program(a_list, b_list):
    """Build the single-core Bass program (same for all cores)."""
    import concourse.tile as tile
    import concourse.mybir as mybir
    from concourse import bacc
    from contextlib import ExitStack

    f32 = mybir.dt.float32
    f16 = mybir.dt.float16
    win = _windows(a_list, b_list)
    # last m-tile where each chunk has any unmasked columns (stop flags)
    last_act = [max(i for i in range(MT) if win[i][0][c] is not None)
                for c in range(2)]

    nc = bacc.Bacc("TRN2", target_bir_lowering=False, debug=False, num_devices=NCORES)

    d_xT = nc.dram_tensor("xT", (128, 2, KO, 512), f16, kind="ExternalInput").ap()
    d_wqT = nc.dram_tensor("wqT", (128, KO, HLOC * DK), f16, kind="ExternalInput").ap()
    d_ktr = nc.dram_tensor("ktr", (HLOC, DK, M), f16, kind="ExternalInput").ap()
    d_v = nc.dram_tensor("v", (HLOC, 128, MT, DK), f16, kind="ExternalInput").ap()
    d_woT = nc.dram_tensor("woT", (128, HLOC, D), f16, kind="ExternalInput").ap()
    d_cosq = nc.dram_tensor("cosq", (DK, T), f16, kind="ExternalInput").ap()
    d_sinq = nc.dram_tensor("sinq", (DK, T), f16, kind="ExternalInput").ap()
    d_posr = nc.dram_tensor("posr", (1, T), f32, kind="ExternalInput").ap()
    d_miota = nc.dram_tensor("miota", (128, MT), f32, kind="ExternalInput").ap()
    d_ones = nc.dram_tensor("ones", (128, 128), f16, kind="ExternalInput").ap()
    d_out = nc.dram_tensor("outT", (D, T), f16, kind="ExternalOutput").ap()

    with tile.TileContext(nc) as tc, ExitStack() as ctx:
        const = ctx.enter_context(tc.tile_pool(name="const", bufs=1))
        big = ctx.enter_context(tc.tile_pool(name="big", bufs=1))
        qpool = ctx.enter_context(tc.tile_pool(name="qpool", bufs=2))
        qtmp = ctx.enter_context(tc.tile_pool(name="qtmp", bufs=2))
        rspool = ctx.enter_context(tc.tile_pool(name="rspool", bufs=2))
        ppool = ctx.enter_context(tc.tile_pool(name="ppool", bufs=4))
        opool = ctx.enter_context(tc.tile_pool(name="opool", bufs=2))
        ostage = ctx.enter_context(tc.tile_pool(name="ostage", bufs=8))
        ps_main = ctx.enter_context(tc.tile_pool(name="ps_main", bufs=2, space="PSUM"))
        ps_acc = ctx.enter_context(tc.tile_pool(name="ps_acc", bufs=1, space="PSUM"))
        ps_q = ctx.enter_context(tc.tile_pool(name="ps_q", bufs=1, space="PSUM"))
        xpool_cm = tc.tile_pool(name="xpool", bufs=1)
        xpool = xpool_cm.__enter__()

        # ---------------- loads (3 independent DGE rings) ----------------
        # Aggregate DMA tops out ~350GB/s; attention can't start before
        # wq+x+cos/sin land, so sync+scalar carry exactly those. gpsimd
        # carries K/V/Wo/mask bulk in consumption order.
        wqT_sb = xpool.tile([128, KO, HLOC * DK], f16, name="wqT_sb")
        nc.sync.dma_start(out=wqT_sb[:, 0:8, :], in_=d_wqT[:, 0:8, :])
        nc.scalar.dma_start(out=wqT_sb[:, 8:16, :], in_=d_wqT[:, 8:16, :])
        xT_sb = xpool.tile([128, 2, KO, 512], f16, name="xT_sb")
        last_xt = None
        for c in range(2):
            for g in range(4):
                ks = slice(g * 4, (g + 1) * 4)
                eng = nc.sync if g % 2 == 0 else nc.scalar
                last_xt = eng.dma_start(out=xT_sb[:, c, ks, :], in_=d_xT[:, c, ks, :])

        # gpsimd ring: ktr0 (first S ~26us), rope tables (rope c0-h0 ~24us),
        # mask consts (first STT), v0 (first PV), then head-1 K/V and Wo
        ktr_sb = []
        v_sb = []
        for h in range(HLOC):
            ktr_sb.append(big.tile([128, M], f16, name=f"ktr_sb{h}"))
            v_sb.append(big.tile([128, MT, DK], f16, name=f"v_sb{h}"))
        nc.gpsimd.dma_start(out=ktr_sb[0][:], in_=d_ktr[0])
        cosq_sb = const.tile([128, T], f16, name="cosq_sb")
        nc.gpsimd.dma_start(out=cosq_sb[:], in_=d_cosq)
        sinq_sb = const.tile([128, T], f16, name="sinq_sb")
        nc.gpsimd.dma_start(out=sinq_sb[:], in_=d_sinq)
        posr_sb = const.tile([128, T], f32, name="posr_sb")
        nc.gpsimd.dma_start(out=posr_sb[:], in_=d_posr.partition_broadcast(128))
        miota_sb = const.tile([128, MT], f32, name="miota_sb")
        nc.gpsimd.dma_start(out=miota_sb[:], in_=d_miota)
        ones_sb = const.tile([128, 128], f16, name="ones_sb")
        nc.gpsimd.dma_start(out=ones_sb[:], in_=d_ones)
        nc.gpsimd.dma_start(out=v_sb[0][:], in_=d_v[0])
        # head-1 K/V and Wo aren't needed until mid-attention: gate them
        # behind the last x slice so x keeps full DMA bandwidth
        from concourse.tile_rust import add_dep_helper
        woT_sb = big.tile([128, HLOC, D], f16, name="woT_sb")
        for g_ in (nc.gpsimd.dma_start(out=ktr_sb[1][:], in_=d_ktr[1]),
                   nc.gpsimd.dma_start(out=v_sb[1][:], in_=d_v[1]),
                   nc.gpsimd.dma_start(out=woT_sb[:], in_=d_woT)):
            add_dep_helper(g_.ins, last_xt.ins, sync=True,
                           reason="stage late loads after xT")

        # rowsum accumulators zeroed up-front (DVE idle during load)
        rsacc = []
        for h in range(HLOC):
            rt = rspool.tile([128, T], f16, name=f"rsacc{h}")
            nc.vector.memset(rt[:], 0.0)
            rsacc.append(rt)

        # ---------------- Q projection + RoPE (h-major) ----------------
        qtr = []
        for h in range(HLOC):
            qps = (ps_acc if h == 0 else ps_q).tile(
                [128, T], f32, tag="acc" if h == 0 else "q", name=f"qps{h}")
            qt = qpool.tile([128, T], f16, tag="qtr", name=f"qtr{h}")
            for c in range(2):
                cs = slice(c * 512, (c + 1) * 512)
                for k in range(KO):
                    nc.tensor.matmul(
                        qps[:, cs],
                        wqT_sb[:, k, h * DK:(h + 1) * DK],
                        xT_sb[:, c, k, :],
                        start=(k == 0), stop=(k == KO - 1),
                    )
                # rope: one fp32->fp16 copy out of PSUM, rest 16-bit DVE ops
                qc = qtmp.tile([128, 512], f16, tag="qc")
                nc.vector.tensor_copy(qc[:], qps[:, cs])
                qrot = qtmp.tile([128, 512], f16, tag="qrot")
                nc.vector.tensor_copy(qrot[0:64, :], qc[64:128, :])
                nc.vector.tensor_copy(qrot[64:128, :], qc[0:64, :])
                t1 = qtmp.tile([128, 512], f16, tag="t1")
                nc.vector.tensor_mul(t1[:], qrot[:], sinq_sb[:, cs])
                t2 = qtmp.tile([128, 512], f16, tag="t2")
                nc.vector.tensor_mul(t2[:], qc[:], cosq_sb[:, cs])
                nc.vector.tensor_add(qt[:, cs], t1[:], t2[:])
            qtr.append(qt)

        xpool_cm.__exit__(None, None, None)  # free xT/wqT SBUF

        # ---------------- attention per head ----------------
        onorm = [None, None]
        oaccs = [None, None]

        def emit_norm(h, rs_pool, rs_tag):
            # partition-reduce rsacc via one ones-matmul (row sums replicated
            # across partitions), fast reciprocal, fold into O
            rs_ps = rs_pool.tile([128, T], f32, tag=rs_tag, name=f"rs_{h}")
            for c in range(2):
                cs = slice(c * 512, (c + 1) * 512)
                nc.tensor.matmul(rs_ps[:, cs], ones_sb[:], rsacc[h][:, cs],
                                 start=True, stop=True)
            rsinv = qtmp.tile([128, T], f32, tag="rsinv")
            nc.vector.reciprocal_approx_fast(out=rsinv[:], in_=rs_ps[:])
            oh = opool.tile([128, T], f16, tag="onorm", name=f"onorm{h}")
            for c in range(2):
                cs = slice(c * 512, (c + 1) * 512)
                nc.vector.tensor_mul(oh[:, cs], oaccs[h][:, cs], rsinv[:, cs])
            onorm[h] = oh

        for h in range(HLOC):
            oacc = (ps_acc if h == 0 else ps_q).tile(
                [128, T], f32, tag="acc" if h == 0 else "q", name=f"oacc{h}")
            oaccs[h] = oacc
            started = [False, False]
            for i in range(MT):
                if h == 1 and i == 2:
                    # h0's normalize emitted 2 tiles into h1's loop so its
                    # rs_ps/reciprocal don't stall the S-tile double-buffer
                    emit_norm(0, ps_main, "mm")
                chunk_lo, lo, b = win[i]
                sps = ps_main.tile([128, T], f32, tag="mm", name=f"s_{h}_{i}")
                for c in range(2):
                    clo = chunk_lo[c]
                    if clo is None:
                        continue
                    nc.tensor.matmul(
                        sps[:, clo:512 * (c + 1)],
                        ktr_sb[h][:, i * 128:(i + 1) * 128],
                        qtr[h][:, clo:512 * (c + 1)],
                        start=True, stop=True,
                    )
                p = ppool.tile([128, T], f16, tag="p")
                nc.scalar.activation(p[:, lo:], sps[:, lo:],
                                     mybir.ActivationFunctionType.Exp, scale=SCALE)
                if b > lo:
                    nc.vector.scalar_tensor_tensor(
                        out=p[:, lo:b], in0=posr_sb[:, lo:b],
                        scalar=miota_sb[:, i:i + 1], in1=p[:, lo:b],
                        op0=mybir.AluOpType.is_ge, op1=mybir.AluOpType.mult,
                    )
                for c in range(2):
                    clo = chunk_lo[c]
                    if clo is None:
                        continue
                    nc.tensor.matmul(
                        oacc[:, clo:512 * (c + 1)],
                        v_sb[h][:, i, :],
                        p[:, clo:512 * (c + 1)],
                        start=not started[c], stop=(i == last_act[c]),
                    )
                    started[c] = True
                nc.vector.tensor_add(rsacc[h][:, lo:], rsacc[h][:, lo:], p[:, lo:])
        # h1's normalize: rs_ps in ps_acc (oacc h0 freed after its normalize)
        # so outproj's first ps_main jps isn't gated on the reciprocal
        emit_norm(1, ps_acc, "acc")

        # ---------------- output projection ----------------
        # jps rotates over 4 PSUM homes (ps_main x2, ps_acc, ps_q -- the
        # accumulators are free post-normalize) so PE runs 4 deep ahead of
        # the copies; each copy is split scalar/vector halves in parallel.
        outT_r = d_out.rearrange("(jo p) t -> p jo t", p=128)
        for j in range(KO):
            r4 = j % 4
            if r4 == 1:
                jps = ps_acc.tile([128, T], f32, tag="acc", name=f"jps{j}")
            elif r4 == 3:
                jps = ps_q.tile([128, T], f32, tag="q", name=f"jps{j}")
            else:
                jps = ps_main.tile([128, T], f32, tag="mm", name=f"jps{j}")
            for c in range(2):
                cs = slice(c * 512, (c + 1) * 512)
                for ho in range(HLOC):
                    nc.tensor.matmul(
                        jps[:, cs],
                        woT_sb[:, ho, j * 128:(j + 1) * 128],
                        onorm[ho][:, cs],
                        start=(ho == 0), stop=(ho == HLOC - 1),
                    )
            ost = ostage.tile([128, T], f16, tag="ost")
            nc.scalar.copy(ost[:, 0:512], jps[:, 0:512])
            nc.vector.tensor_copy(ost[:, 512:], jps[:, 512:])
            if j >= KO - 4:
                # drain the last tiles as parallel half-DMAs on two rings
                nc.sync.dma_start(out=outT_r[:, j, 0:512], in_=ost[:, 0:512])
                nc.gpsimd.dma_start(out=outT_r[:, j, 512:], in_=ost[:, 512:])
            else:
                dma_eng = (nc.sync, nc.gpsimd, nc.scalar)[j % 3]
                dma_eng.dma_start(out=outT_r[:, j, :], in_=ost[:])

    nc.compile()
    return nc


def _prep(inputs):
    """Host-side prep shared by kernel() and test harnesses."""
    x = np.asarray(inputs["x"], dtype=np.float32)
    k_ctx = np.asarray(inputs["k_ctx"], dtype=np.float32)
    v_ctx = np.asarray(inputs["v_ctx"], dtype=np.float32)
    W_q = np.asarray(inputs["W_q"], dtype=np.float32)
    W_o = np.asarray(inputs["W_o"], dtype=np.float32)
    pos_np = np.asarray(inputs["positions"]).astype(np.int64)
    pctx_np = np.asarray(inputs["p_ctx"]).astype(np.int64)

    perm = np.argsort(pos_np, kind="stable")
    ps = pos_np[perm]
    xs_T = x[perm].T.astype(np.float16)                                  # [D, T]
    xT = np.ascontiguousarray(
        xs_T.reshape(KO, 128, 2, 512).transpose(1, 2, 0, 3))             # [128,2,KO,512]
    k_rope = _host_rope_k(k_ctx, pctx_np).astype(np.float16)
    cosq, sinq = _host_q_tables(ps)
    cosq = cosq.astype(np.float16)
    sinq = sinq.astype(np.float16)
    posr = ps.astype(np.float32).reshape(1, T)
    miota = (np.arange(MT)[None, :] * 128 + np.arange(128)[:, None]).astype(np.float32)
    a_list = [int(np.searchsorted(ps, 128 * i, side="left")) for i in range(MT)]
    b_list = [int(np.searchsorted(ps, 128 * i + 127, side="left")) for i in range(MT)]

    in_maps = []
    for c in range(NCORES):
        hs = slice(c * HLOC * DK, (c + 1) * HLOC * DK)
        heads = range(c * HLOC, (c + 1) * HLOC)
        wq = W_q[hs, :].T.reshape(KO, 128, HLOC * DK).astype(np.float16)
        wo = W_o[:, hs].T.reshape(HLOC, 128, D).astype(np.float16)
        vv = v_ctx.transpose(1, 0, 2)[c * HLOC:(c + 1) * HLOC].astype(np.float16)
        in_maps.append({
            "xT": xT,
            "wqT": np.ascontiguousarray(wq.transpose(1, 0, 2)),
            "ktr": np.ascontiguousarray(np.stack([k_rope[:, h, :].T for h in heads])),
            "v": np.ascontiguousarray(vv.reshape(HLOC, MT, 128, DK).transpose(0, 2, 1, 3)),
            "woT": np.ascontiguousarray(wo.transpose(1, 0, 2)),
            "cosq": cosq, "sinq": sinq, "posr": posr, "miota": miota,
            "ones": np.ones((128, 128), dtype=np.float16),
        })
    return perm, a_list, b_list, in_maps


def kernel(x, k_ctx, v_ctx, W_q, W_o, positions, p_ctx):
    from concourse.bass_utils import run_bass_kernel_spmd

    inputs = dict(x=x, k_ctx=k_ctx, v_ctx=v_ctx, W_q=W_q, W_o=W_o,
                  positions=positions, p_ctx=p_ctx)
    perm, a_list, b_list, in_maps = _prep(inputs)

    key = (tuple(a_list), tuple(b_list))
    if key not in _PROGRAM_CACHE:
        _PROGRAM_CACHE[key] = _build_program(a_list, b_list)
    nc = _PROGRAM_CACHE[key]

    r = run_bass_kernel_spmd(nc, in_maps, core_ids=list(range(NCORES)))

    acc = np.zeros((D, T), dtype=np.float64)
    for c in range(NCORES):
        acc += r.results[c]["outT"].astype(np.float64)
    out_sorted = acc.T.astype(np.float32)
    out = np.empty_like(out_sorted)
    out[perm] = out_sorted
    return out.astype(np.float32)


if __name__ == "__main__":
    import importlib.util
    spec = importlib.util.spec_from_file_location("reference", "/root/problem/reference.py")
    ref = importlib.util.module_from_spec(spec)
    spec.loader.exec_module(ref)
    inputs = {k: np.asarray(v) for k, v in ref.setup_inputs().items()}
    expected = np.asarray(ref.reference(**inputs))
    got = kernel(**inputs)
    err = np.abs(got - expected)
    print("absmax err:", err.max(), "rel:", err.max() / np.abs(expected).max())


# revision 21
# speedup vs baseline: 1.0158x; 1.0158x over previous
"""CacheAwareMHA TRN2 kernel: 8-core head-sharded attention (fp16 PE path).

Strategy (v2):
  - Shard heads (16) across 8 cores: 2 heads/core. Each core holds W_q/W_o
    column/row shards and its heads' K/V slices; partial outputs summed on host.
  - ALL matmuls in float16 (1 cycle/col on PE vs ~3 for fp32); PSUM stays fp32.
    Tolerance is 2e-2; fp16 keeps rel err ~1e-3.
  - S^T layout [m partitions, t free] so softmax weights feed P@V directly.
  - Queries sorted by position on host -> causal mask is a per-(m-tile)
    t-prefix; windows are exact (aligned down to 16 cols for 32B ISA starts);
    the ~16-48 col partial band is masked with one scalar_tensor_tensor.
  - exp without max-subtraction (logits ~N(0,1); fp16 P max ~e^7 ok, row sums
    ~5e3 < fp16 max 65504).
  - Row sums on the DVE (in-place fp16 accumulate per m-tile, 4x mode) --
    frees ~20us of PE ones-matmul streaming.
  - Loads split across 4 DGE rings (vector: tables; sync+scalar: wq/x
    fine-grained; gpsimd: k/v/wo bulk) so Qproj starts at ~4us.
  - Output projection emitted fp16; partials summed on host.
"""
import sys
import math

import numpy as np

for _p in ("/opt/trn_rl_repo", "/opt/pypackages"):
    if _p not in sys.path:
        sys.path.append(_p)

T, D, H, DK, M = 1024, 2048, 16, 128, 4096
NCORES = 8
HLOC = H // NCORES  # heads per core
KO = D // 128       # 16 contraction tiles for projections
MT = M // 128       # 32 m-tiles
ROPE_BASE = 10000.0
SCALE = 1.0 / math.sqrt(DK)

_PROGRAM_CACHE = {}


def _host_rope_k(k, pos):
    """Apply RoPE to cached keys on host (fp64 tables). k: [M, h, DK]."""
    inv = 1.0 / (ROPE_BASE ** (np.arange(0, DK, 2, dtype=np.float64) / DK))
    th = pos[:, None].astype(np.float64) * inv[None, :]
    cos = np.concatenate([np.cos(th), np.cos(th)], -1)[:, None, :]
    sin = np.concatenate([np.sin(th), np.sin(th)], -1)[:, None, :]
    t1, t2 = k[..., :64], k[..., 64:]
    rot = np.concatenate([-t2, t1], -1)
    return (k.astype(np.float64) * cos + rot.astype(np.float64) * sin).astype(np.float32)


def _host_q_tables(pos_sorted):
    """cos / sign-baked sin tables in Q^T layout [DK, T] (fp32)."""
    inv = 1.0 / (ROPE_BASE ** (np.arange(0, DK, 2, dtype=np.float64) / DK))
    th = pos_sorted[None, :].astype(np.float64) * inv[:, None]      # [64, T]
    cos = np.cos(th)
    sin = np.sin(th)
    cosT = np.concatenate([cos, cos], 0).astype(np.float32)          # [128, T]
    sinT = np.concatenate([-sin, sin], 0).astype(np.float32)         # sign baked
    return cosT, sinT


def _windows(a_list, b_list):
    """Per m-tile: (chunk_lo[2], lo, stt_hi).

    lo = exact causal window start aligned down to 16 (32B fp16 matmul
    operand starts; PSUM fp32 outs need 8 -> 16 covers both). chunk_lo[c]
    is the start col for S/PV matmuls in 512-col PSUM-bank chunk c
    (None = chunk fully masked)."""
    out = []
    for i in range(MT):
        lo = a_list[i] & ~15
        chunk_lo = []
        for c in range(2):
            clo = max(512 * c, lo)
            chunk_lo.append(clo if clo < 512 * (c + 1) else None)
        out.append((chunk_lo, lo, b_list[i]))
    return out


def _build_program(a_list, b_list):
    """Build the single-core Bass program (same for all cores)."""
    import concourse.tile as tile
    import concourse.mybir as mybir
    from concourse import bacc
    from contextlib import ExitStack

    f32 = mybir.dt.float32
    f16 = mybir.dt.float16
    win = _windows(a_list, b_list)
    # last m-tile where each chunk has any unmasked columns (stop flags)
    last_act = [max(i for i in range(MT) if win[i][0][c] is not None)
                for c in range(2)]

    nc = bacc.Bacc("TRN2", target_bir_lowering=False, debug=False, num_devices=NCORES)

    d_xT = nc.dram_tensor("xT", (128, 2, KO, 512), f16, kind="ExternalInput").ap()
    d_wqT = nc.dram_tensor("wqT", (128, KO, HLOC * DK), f16, kind="ExternalInput").ap()
    d_ktr = nc.dram_tensor("ktr", (HLOC, DK, M), f16, kind="ExternalInput").ap()
    d_v = nc.dram_tensor("v", (HLOC, 128, MT, DK), f16, kind="ExternalInput").ap()
    d_woT = nc.dram_tensor("woT", (128, HLOC, D), f16, kind="ExternalInput").ap()
    d_cosq = nc.dram_tensor("cosq", (DK, T), f16, kind="ExternalInput").ap()
    d_sinq = nc.dram_tensor("sinq", (DK, T), f16, kind="ExternalInput").ap()
    d_posr = nc.dram_tensor("posr", (1, T), f32, kind="ExternalInput").ap()
    d_miota = nc.dram_tensor("miota", (128, MT), f32, kind="ExternalInput").ap()
    d_ones = nc.dram_tensor("ones", (128, 128), f16, kind="ExternalInput").ap()
    d_out = nc.dram_tensor("outT", (D, T), f16, kind="ExternalOutput").ap()

    with tile.TileContext(nc) as tc, ExitStack() as ctx:
        const = ctx.enter_context(tc.tile_pool(name="const", bufs=1))
        big = ctx.enter_context(tc.tile_pool(name="big", bufs=1))
        qpool = ctx.enter_context(tc.tile_pool(name="qpool", bufs=2))
        qtmp = ctx.enter_context(tc.tile_pool(name="qtmp", bufs=2))
        rspool = ctx.enter_context(tc.tile_pool(name="rspool", bufs=2))
        ppool = ctx.enter_context(tc.tile_pool(name="ppool", bufs=6))
        opool = ctx.enter_context(tc.tile_pool(name="opool", bufs=2))
        ostage = ctx.enter_context(tc.tile_pool(name="ostage", bufs=8))
        ps_main = ctx.enter_context(tc.tile_pool(name="ps_main", bufs=2, space="PSUM"))
        ps_acc = ctx.enter_context(tc.tile_pool(name="ps_acc", bufs=1, space="PSUM"))
        ps_q = ctx.enter_context(tc.tile_pool(name="ps_q", bufs=1, space="PSUM"))
        xpool_cm = tc.tile_pool(name="xpool", bufs=1)
        xpool = xpool_cm.__enter__()

        # ---------------- loads (3 independent DGE rings) ----------------
        # Aggregate DMA tops out ~350GB/s; attention can't start before
        # wq+x+cos/sin land, so sync+scalar carry exactly those. gpsimd
        # carries K/V/Wo/mask bulk in consumption order.
        wqT_sb = xpool.tile([128, KO, HLOC * DK], f16, name="wqT_sb")
        nc.sync.dma_start(out=wqT_sb[:, 0:8, :], in_=d_wqT[:, 0:8, :])
        nc.scalar.dma_start(out=wqT_sb[:, 8:16, :], in_=d_wqT[:, 8:16, :])
        xT_sb = xpool.tile([128, 2, KO, 512], f16, name="xT_sb")
        last_xt = None
        for c in range(2):
            for g in range(4):
                ks = slice(g * 4, (g + 1) * 4)
                eng = nc.sync if g % 2 == 0 else nc.scalar
                last_xt = eng.dma_start(out=xT_sb[:, c, ks, :], in_=d_xT[:, c, ks, :])

        # gpsimd ring in need-order: rope tables (rope c0 ~22us), ktr0 (first
        # S ~30us), mask consts (first STT, ppool slack tolerates ~+5us),
        # v0 (first PV, same slack)
        ktr_sb = []
        v_sb = []
        for h in range(HLOC):
            ktr_sb.append(big.tile([128, M], f16, name=f"ktr_sb{h}"))
            v_sb.append(big.tile([128, MT, DK], f16, name=f"v_sb{h}"))
        cosq_sb = const.tile([128, T], f16, name="cosq_sb")
        nc.gpsimd.dma_start(out=cosq_sb[:], in_=d_cosq)
        sinq_sb = const.tile([128, T], f16, name="sinq_sb")
        nc.gpsimd.dma_start(out=sinq_sb[:], in_=d_sinq)
        nc.gpsimd.dma_start(out=ktr_sb[0][:], in_=d_ktr[0])
        posr_sb = const.tile([128, T], f32, name="posr_sb")
        nc.gpsimd.dma_start(out=posr_sb[:], in_=d_posr.partition_broadcast(128))
        miota_sb = const.tile([128, MT], f32, name="miota_sb")
        nc.gpsimd.dma_start(out=miota_sb[:], in_=d_miota)
        ones_sb = const.tile([128, 128], f16, name="ones_sb")
        nc.gpsimd.dma_start(out=ones_sb[:], in_=d_ones)
        nc.gpsimd.dma_start(out=v_sb[0][:], in_=d_v[0])
        # head-1 K/V and Wo aren't needed until mid-attention: gate them
        # behind the last x slice so x keeps full DMA bandwidth
        from concourse.tile_rust import add_dep_helper
        woT_sb = big.tile([128, HLOC, D], f16, name="woT_sb")
        for g_ in (nc.gpsimd.dma_start(out=ktr_sb[1][:], in_=d_ktr[1]),
                   nc.gpsimd.dma_start(out=v_sb[1][:], in_=d_v[1]),
                   nc.gpsimd.dma_start(out=woT_sb[:], in_=d_woT)):
            add_dep_helper(g_.ins, last_xt.ins, sync=True,
                           reason="stage late loads after xT")

        # rowsum accumulators zeroed up-front (DVE idle during load)
        rsacc = []
        for h in range(HLOC):
            rt = rspool.tile([128, T], f16, name=f"rsacc{h}")
            nc.vector.memset(rt[:], 0.0)
            rsacc.append(rt)

        # ---------------- Q projection + RoPE (h-major) ----------------
        qtr = []
        for h in range(HLOC):
            qps = (ps_acc if h == 0 else ps_q).tile(
                [128, T], f32, tag="acc" if h == 0 else "q", name=f"qps{h}")
            qt = qpool.tile([128, T], f16, tag="qtr", name=f"qtr{h}")
            for c in range(2):
                cs = slice(c * 512, (c + 1) * 512)
                for k in range(KO):
                    nc.tensor.matmul(
                        qps[:, cs],
                        wqT_sb[:, k, h * DK:(h + 1) * DK],
                        xT_sb[:, c, k, :],
                        start=(k == 0), stop=(k == KO - 1),
                    )
                # rope: one fp32->fp16 copy out of PSUM, rest 16-bit DVE ops
                qc = qtmp.tile([128, 512], f16, tag="qc")
                nc.vector.tensor_copy(qc[:], qps[:, cs])
                qrot = qtmp.tile([128, 512], f16, tag="qrot")
                nc.vector.tensor_copy(qrot[0:64, :], qc[64:128, :])
                nc.vector.tensor_copy(qrot[64:128, :], qc[0:64, :])
                t1 = qtmp.tile([128, 512], f16, tag="t1")
                nc.vector.tensor_mul(t1[:], qrot[:], sinq_sb[:, cs])
                t2 = qtmp.tile([128, 512], f16, tag="t2")
                nc.vector.tensor_mul(t2[:], qc[:], cosq_sb[:, cs])
                nc.vector.tensor_add(qt[:, cs], t1[:], t2[:])
            qtr.append(qt)

        xpool_cm.__exit__(None, None, None)  # free xT/wqT SBUF

        # ---------------- attention per head ----------------
        onorm = [None, None]
        oaccs = [None, None]

        def emit_norm(h, rs_pool, rs_tag):
            # partition-reduce rsacc via one ones-matmul (row sums replicated
            # across partitions), fast reciprocal, fold into O
            rs_ps = rs_pool.tile([128, T], f32, tag=rs_tag, name=f"rs_{h}")
            for c in range(2):
                cs = slice(c * 512, (c + 1) * 512)
                nc.tensor.matmul(rs_ps[:, cs], ones_sb[:], rsacc[h][:, cs],
                                 start=True, stop=True)
            rsinv = qtmp.tile([128, T], f32, tag="rsinv")
            nc.vector.reciprocal_approx_fast(out=rsinv[:], in_=rs_ps[:])
            oh = opool.tile([128, T], f16, tag="onorm", name=f"onorm{h}")
            for c in range(2):
                cs = slice(c * 512, (c + 1) * 512)
                nc.vector.tensor_mul(oh[:, cs], oaccs[h][:, cs], rsinv[:, cs])
            onorm[h] = oh

        for h in range(HLOC):
            oacc = (ps_acc if h == 0 else ps_q).tile(
                [128, T], f32, tag="acc" if h == 0 else "q", name=f"oacc{h}")
            oaccs[h] = oacc
            started = [False, False]
            for i in range(MT):
                if h == 1 and i == 2:
                    # h0's normalize emitted 2 tiles into h1's loop so its
                    # rs_ps/reciprocal don't stall the S-tile double-buffer
                    emit_norm(0, ps_main, "mm")
                chunk_lo, lo, b = win[i]
                sps = ps_main.tile([128, T], f32, tag="mm", name=f"s_{h}_{i}")
                for c in range(2):
                    clo = chunk_lo[c]
                    if clo is None:
                        continue
                    nc.tensor.matmul(
                        sps[:, clo:512 * (c + 1)],
                        ktr_sb[h][:, i * 128:(i + 1) * 128],
                        qtr[h][:, clo:512 * (c + 1)],
                        start=True, stop=True,
                    )
                p = ppool.tile([128, T], f16, tag="p")
                nc.scalar.activation(p[:, lo:], sps[:, lo:],
                                     mybir.ActivationFunctionType.Exp, scale=SCALE)
                if b > lo:
                    nc.vector.scalar_tensor_tensor(
                        out=p[:, lo:b], in0=posr_sb[:, lo:b],
                        scalar=miota_sb[:, i:i + 1], in1=p[:, lo:b],
                        op0=mybir.AluOpType.is_ge, op1=mybir.AluOpType.mult,
                    )
                for c in range(2):
                    clo = chunk_lo[c]
                    if clo is None:
                        continue
                    nc.tensor.matmul(
                        oacc[:, clo:512 * (c + 1)],
                        v_sb[h][:, i, :],
                        p[:, clo:512 * (c + 1)],
                        start=not started[c], stop=(i == last_act[c]),
                    )
                    started[c] = True
                nc.vector.tensor_add(rsacc[h][:, lo:], rsacc[h][:, lo:], p[:, lo:])
        # h1's normalize: rs_ps in ps_acc (oacc h0 freed after its normalize)
        # so outproj's first ps_main jps isn't gated on the reciprocal
        emit_norm(1, ps_acc, "acc")

        # ---------------- output projection ----------------
        # jps rotates over 4 PSUM homes (ps_main x2, ps_acc, ps_q -- the
        # accumulators are free post-normalize) so PE runs 4 deep ahead of
        # the copies; each copy is split scalar/vector halves in parallel.
        outT_r = d_out.rearrange("(jo p) t -> p jo t", p=128)
        for j in range(KO):
            r4 = j % 4
            if r4 == 1:
                jps = ps_acc.tile([128, T], f32, tag="acc", name=f"jps{j}")
            elif r4 == 3:
                jps = ps_q.tile([128, T], f32, tag="q", name=f"jps{j}")
            else:
                jps = ps_main.tile([128, T], f32, tag="mm", name=f"jps{j}")
            for c in range(2):
                cs = slice(c * 512, (c + 1) * 512)
                for ho in range(HLOC):
                    nc.tensor.matmul(
                        jps[:, cs],
                        woT_sb[:, ho, j * 128:(j + 1) * 128],
                        onorm[ho][:, cs],
                        start=(ho == 0), stop=(ho == HLOC - 1),
                    )
            ost = ostage.tile([128, T], f16, tag="ost")
            nc.scalar.copy(ost[:, 0:512], jps[:, 0:512])
            nc.vector.tensor_copy(ost[:, 512:], jps[:, 512:])
            if j >= KO - 4:
                # drain the last tiles as parallel half-DMAs on two rings
                nc.sync.dma_start(out=outT_r[:, j, 0:512], in_=ost[:, 0:512])
                nc.gpsimd.dma_start(out=outT_r[:, j, 512:], in_=ost[:, 512:])
            else:
                dma_eng = (nc.sync, nc.gpsimd, nc.scalar)[j % 3]
                dma_eng.dma_start(out=outT_r[:, j, :], in_=ost[:])

    nc.compile()
    return nc


def _prep(inputs):
    """Host-side prep shared by kernel() and test harnesses."""
    x = np.asarray(inputs["x"], dtype=np.float32)
    k_ctx = np.asarray(inputs["k_ctx"], dtype=np.float32)
    v_ctx = np.asarray(inputs["v_ctx"], dtype=np.float32)
    W_q = np.asarray(inputs["W_q"], dtype=np.float32)
    W_o = np.asarray(inputs["W_o"], dtype=np.float32)
    pos_np = np.asarray(inputs["positions"]).astype(np.int64)
    pctx_np = np.asarray(inputs["p_ctx"]).astype(np.int64)

    perm = np.argsort(pos_np, kind="stable")
    ps = pos_np[perm]
    xs_T = x[perm].T.astype(np.float16)                                  # [D, T]
    xT = np.ascontiguousarray(
        xs_T.reshape(KO, 128, 2, 512).transpose(1, 2, 0, 3))             # [128,2,KO,512]
    k_rope = _host_rope_k(k_ctx, pctx_np).astype(np.float16)
    cosq, sinq = _host_q_tables(ps)
    cosq = cosq.astype(np.float16)
    sinq = sinq.astype(np.float16)
    posr = ps.astype(np.float32).reshape(1, T)
    miota = (np.arange(MT)[None, :] * 128 + np.arange(128)[:, None]).astype(np.float32)
    a_list = [int(np.searchsorted(ps, 128 * i, side="left")) for i in range(MT)]
    b_list = [int(np.searchsorted(ps, 128 * i + 127, side="left")) for i in range(MT)]

    in_maps = []
    for c in range(NCORES):
        hs = slice(c * HLOC * DK, (c + 1) * HLOC * DK)
        heads = range(c * HLOC, (c + 1) * HLOC)
        wq = W_q[hs, :].T.reshape(KO, 128, HLOC * DK).astype(np.float16)
        wo = W_o[:, hs].T.reshape(HLOC, 128, D).astype(np.float16)
        vv = v_ctx.transpose(1, 0, 2)[c * HLOC:(c + 1) * HLOC].astype(np.float16)
        in_maps.append({
            "xT": xT,
            "wqT": np.ascontiguousarray(wq.transpose(1, 0, 2)),
            "ktr": np.ascontiguousarray(np.stack([k_rope[:, h, :].T for h in heads])),
            "v": np.ascontiguousarray(vv.reshape(HLOC, MT, 128, DK).transpose(0, 2, 1, 3)),
            "woT": np.ascontiguousarray(wo.transpose(1, 0, 2)),
            "cosq": cosq, "sinq": sinq, "posr": posr, "miota": miota,
            "ones": np.ones((128, 128), dtype=np.float16),
        })
    return perm, a_list, b_list, in_maps


def kernel(x, k_ctx, v_ctx, W_q, W_o, positions, p_ctx):
    from concourse.bass_utils import run_bass_kernel_spmd

    inputs = dict(x=x, k_ctx=k_ctx, v_ctx=v_ctx, W_q=W_q, W_o=W_o,
                  positions=positions, p_ctx=p_ctx)
    perm, a_list, b_list, in_maps = _prep(inputs)

    key = (tuple(a_list), tuple(b_list))
    if key not in _PROGRAM_CACHE:
        _PROGRAM_CACHE[key] = _build_program(a_list, b_list)
    nc = _PROGRAM_CACHE[key]

    r = run_bass_kernel_spmd(nc, in_maps, core_ids=list(range(NCORES)))

    acc = np.zeros((D, T), dtype=np.float64)
    for c in range(NCORES):
        acc += r.results[c]["outT"].astype(np.float64)
    out_sorted = acc.T.astype(np.float32)
    out = np.empty_like(out_sorted)
    out[perm] = out_sorted
    return out.astype(np.float32)


if __name__ == "__main__":
    import importlib.util
    spec = importlib.util.spec_from_file_location("reference", "/root/problem/reference.py")
    ref = importlib.util.module_from_spec(spec)
    spec.loader.exec_module(ref)
    inputs = {k: np.asarray(v) for k, v in ref.setup_inputs().items()}
    expected = np.asarray(ref.reference(**inputs))
    got = kernel(**inputs)
    err = np.abs(got - expected)
    print("absmax err:", err.max(), "rel:", err.max() / np.abs(expected).max())


# revision 25
# speedup vs baseline: 1.0403x; 1.0242x over previous
"""CacheAwareMHA TRN2 kernel: 8-core head-sharded attention (fp16 PE path).

Strategy (v2):
  - Shard heads (16) across 8 cores: 2 heads/core. Each core holds W_q/W_o
    column/row shards and its heads' K/V slices; partial outputs summed on host.
  - ALL matmuls in float16 (1 cycle/col on PE vs ~3 for fp32); PSUM stays fp32.
    Tolerance is 2e-2; fp16 keeps rel err ~1e-3.
  - S^T layout [m partitions, t free] so softmax weights feed P@V directly.
  - Queries sorted by position on host -> causal mask is a per-(m-tile)
    t-prefix; windows are exact (aligned down to 16 cols for 32B ISA starts);
    the ~16-48 col partial band is masked with one scalar_tensor_tensor.
  - exp without max-subtraction (logits ~N(0,1); fp16 P max ~e^7 ok, row sums
    ~5e3 < fp16 max 65504).
  - Row sums on the DVE (in-place fp16 accumulate per m-tile, 4x mode) --
    frees ~20us of PE ones-matmul streaming.
  - Loads split across 4 DGE rings (vector: tables; sync+scalar: wq/x
    fine-grained; gpsimd: k/v/wo bulk) so Qproj starts at ~4us.
  - Output projection emitted fp16; partials summed on host.
"""
import sys
import math

import numpy as np

for _p in ("/opt/trn_rl_repo", "/opt/pypackages"):
    if _p not in sys.path:
        sys.path.append(_p)

T, D, H, DK, M = 1024, 2048, 16, 128, 4096
NCORES = 8
HLOC = H // NCORES  # heads per core
KO = D // 128       # 16 contraction tiles for projections
MT = M // 128       # 32 m-tiles
ROPE_BASE = 10000.0
SCALE = 1.0 / math.sqrt(DK)

_PROGRAM_CACHE = {}


def _host_rope_k(k, pos):
    """Apply RoPE to cached keys on host (fp64 tables). k: [M, h, DK]."""
    inv = 1.0 / (ROPE_BASE ** (np.arange(0, DK, 2, dtype=np.float64) / DK))
    th = pos[:, None].astype(np.float64) * inv[None, :]
    cos = np.concatenate([np.cos(th), np.cos(th)], -1)[:, None, :]
    sin = np.concatenate([np.sin(th), np.sin(th)], -1)[:, None, :]
    t1, t2 = k[..., :64], k[..., 64:]
    rot = np.concatenate([-t2, t1], -1)
    return (k.astype(np.float64) * cos + rot.astype(np.float64) * sin).astype(np.float32)


def _host_q_tables(pos_sorted):
    """cos / sign-baked sin tables in Q^T layout [DK, T] (fp32)."""
    inv = 1.0 / (ROPE_BASE ** (np.arange(0, DK, 2, dtype=np.float64) / DK))
    th = pos_sorted[None, :].astype(np.float64) * inv[:, None]      # [64, T]
    cos = np.cos(th)
    sin = np.sin(th)
    cosT = np.concatenate([cos, cos], 0).astype(np.float32)          # [128, T]
    sinT = np.concatenate([-sin, sin], 0).astype(np.float32)         # sign baked
    return cosT, sinT


def _windows(a_list, b_list):
    """Per m-tile: (chunk_lo[2], lo, stt_hi).

    lo = exact causal window start aligned down to 16 (32B fp16 matmul
    operand starts; PSUM fp32 outs need 8 -> 16 covers both). chunk_lo[c]
    is the start col for S/PV matmuls in 512-col PSUM-bank chunk c
    (None = chunk fully masked)."""
    out = []
    for i in range(MT):
        lo = a_list[i] & ~15
        chunk_lo = []
        for c in range(2):
            clo = max(512 * c, lo)
            chunk_lo.append(clo if clo < 512 * (c + 1) else None)
        out.append((chunk_lo, lo, b_list[i]))
    return out


def _build_program(a_list, b_list):
    """Build the single-core Bass program (same for all cores)."""
    import concourse.tile as tile
    import concourse.mybir as mybir
    from concourse import bacc
    from contextlib import ExitStack

    f32 = mybir.dt.float32
    f16 = mybir.dt.float16
    win = _windows(a_list, b_list)
    # last m-tile where each chunk has any unmasked columns (stop flags)
    last_act = [max(i for i in range(MT) if win[i][0][c] is not None)
                for c in range(2)]

    nc = bacc.Bacc("TRN2", target_bir_lowering=False, debug=False, num_devices=NCORES)

    d_xT = nc.dram_tensor("xT", (128, 2, KO, 512), f16, kind="ExternalInput").ap()
    d_wqT = nc.dram_tensor("wqT", (128, KO, HLOC * DK), f16, kind="ExternalInput").ap()
    d_ktr = nc.dram_tensor("ktr", (HLOC, DK, M), f16, kind="ExternalInput").ap()
    d_v = nc.dram_tensor("v", (HLOC, 128, MT, DK), f16, kind="ExternalInput").ap()
    d_woT = nc.dram_tensor("woT", (128, HLOC, D), f16, kind="ExternalInput").ap()
    d_cosq = nc.dram_tensor("cosq", (DK, T), f16, kind="ExternalInput").ap()
    d_sinq = nc.dram_tensor("sinq", (DK, T), f16, kind="ExternalInput").ap()
    d_posr = nc.dram_tensor("posr", (1, T), f32, kind="ExternalInput").ap()
    d_miota = nc.dram_tensor("miota", (128, MT), f32, kind="ExternalInput").ap()
    d_ones = nc.dram_tensor("ones", (128, 128), f16, kind="ExternalInput").ap()
    d_out = nc.dram_tensor("outT", (D, T), f16, kind="ExternalOutput").ap()

    with tile.TileContext(nc) as tc, ExitStack() as ctx:
        const = ctx.enter_context(tc.tile_pool(name="const", bufs=1))
        big = ctx.enter_context(tc.tile_pool(name="big", bufs=1))
        qpool = ctx.enter_context(tc.tile_pool(name="qpool", bufs=2))
        qtmp = ctx.enter_context(tc.tile_pool(name="qtmp", bufs=2))
        rspool = ctx.enter_context(tc.tile_pool(name="rspool", bufs=2))
        ppool = ctx.enter_context(tc.tile_pool(name="ppool", bufs=8))
        opool = ctx.enter_context(tc.tile_pool(name="opool", bufs=2))
        ostage = ctx.enter_context(tc.tile_pool(name="ostage", bufs=8))
        ps_main = ctx.enter_context(tc.tile_pool(name="ps_main", bufs=2, space="PSUM"))
        ps_acc = ctx.enter_context(tc.tile_pool(name="ps_acc", bufs=1, space="PSUM"))
        ps_q = ctx.enter_context(tc.tile_pool(name="ps_q", bufs=1, space="PSUM"))
        xpool_cm = tc.tile_pool(name="xpool", bufs=1)
        xpool = xpool_cm.__enter__()

        # ---------------- loads (3 independent DGE rings) ----------------
        # Aggregate DMA tops out ~350GB/s; attention can't start before
        # wq+x+cos/sin land, so sync+scalar carry exactly those. gpsimd
        # carries K/V/Wo/mask bulk in consumption order.
        wqT_sb = xpool.tile([128, KO, HLOC * DK], f16, name="wqT_sb")
        nc.sync.dma_start(out=wqT_sb[:, 0:8, :], in_=d_wqT[:, 0:8, :])
        nc.scalar.dma_start(out=wqT_sb[:, 8:16, :], in_=d_wqT[:, 8:16, :])
        xT_sb = xpool.tile([128, 2, KO, 512], f16, name="xT_sb")
        last_xt = None
        for c in range(2):
            for g in range(4):
                ks = slice(g * 4, (g + 1) * 4)
                eng = nc.sync if g % 2 == 0 else nc.scalar
                last_xt = eng.dma_start(out=xT_sb[:, c, ks, :], in_=d_xT[:, c, ks, :])

        # gpsimd ring: only rope tables + ktr0 compete with x for bandwidth
        # (attention literally can't start without them). Everything else is
        # gated behind the last x slice -- the 8-deep ppool lets the exp
        # stream run ~6 tiles ahead of the first STT/PV that need posr/v0.
        ktr_sb = []
        v_sb = []
        for h in range(HLOC):
            ktr_sb.append(big.tile([128, M], f16, name=f"ktr_sb{h}"))
            v_sb.append(big.tile([128, MT, DK], f16, name=f"v_sb{h}"))
        cosq_sb = const.tile([128, T], f16, name="cosq_sb")
        nc.gpsimd.dma_start(out=cosq_sb[:], in_=d_cosq)
        sinq_sb = const.tile([128, T], f16, name="sinq_sb")
        nc.gpsimd.dma_start(out=sinq_sb[:], in_=d_sinq)
        nc.gpsimd.dma_start(out=ktr_sb[0][:], in_=d_ktr[0])
        posr_sb = const.tile([128, T], f32, name="posr_sb")
        miota_sb = const.tile([128, MT], f32, name="miota_sb")
        ones_sb = const.tile([128, 128], f16, name="ones_sb")
        woT_sb = big.tile([128, HLOC, D], f16, name="woT_sb")
        from concourse.tile_rust import add_dep_helper
        for g_ in (nc.gpsimd.dma_start(out=posr_sb[:],
                                       in_=d_posr.partition_broadcast(128)),
                   nc.gpsimd.dma_start(out=miota_sb[:], in_=d_miota),
                   nc.gpsimd.dma_start(out=ones_sb[:], in_=d_ones),
                   nc.gpsimd.dma_start(out=v_sb[0][:], in_=d_v[0]),
                   nc.gpsimd.dma_start(out=ktr_sb[1][:], in_=d_ktr[1]),
                   nc.gpsimd.dma_start(out=v_sb[1][:], in_=d_v[1]),
                   nc.gpsimd.dma_start(out=woT_sb[:], in_=d_woT)):
            add_dep_helper(g_.ins, last_xt.ins, sync=True,
                           reason="stage late loads after xT")

        # rowsum accumulators zeroed up-front (DVE idle during load)
        rsacc = []
        for h in range(HLOC):
            rt = rspool.tile([128, T], f16, name=f"rsacc{h}")
            nc.vector.memset(rt[:], 0.0)
            rsacc.append(rt)

        # ---------------- Q projection + RoPE (h-major) ----------------
        qtr = []
        for h in range(HLOC):
            qps = (ps_acc if h == 0 else ps_q).tile(
                [128, T], f32, tag="acc" if h == 0 else "q", name=f"qps{h}")
            qt = qpool.tile([128, T], f16, tag="qtr", name=f"qtr{h}")
            for c in range(2):
                cs = slice(c * 512, (c + 1) * 512)
                for k in range(KO):
                    nc.tensor.matmul(
                        qps[:, cs],
                        wqT_sb[:, k, h * DK:(h + 1) * DK],
                        xT_sb[:, c, k, :],
                        start=(k == 0), stop=(k == KO - 1),
                    )
                # rope: one fp32->fp16 copy out of PSUM, rest 16-bit DVE ops
                qc = qtmp.tile([128, 512], f16, tag="qc")
                nc.vector.tensor_copy(qc[:], qps[:, cs])
                qrot = qtmp.tile([128, 512], f16, tag="qrot")
                nc.vector.tensor_copy(qrot[0:64, :], qc[64:128, :])
                nc.vector.tensor_copy(qrot[64:128, :], qc[0:64, :])
                t1 = qtmp.tile([128, 512], f16, tag="t1")
                nc.vector.tensor_mul(t1[:], qrot[:], sinq_sb[:, cs])
                t2 = qtmp.tile([128, 512], f16, tag="t2")
                nc.vector.tensor_mul(t2[:], qc[:], cosq_sb[:, cs])
                nc.vector.tensor_add(qt[:, cs], t1[:], t2[:])
            qtr.append(qt)

        xpool_cm.__exit__(None, None, None)  # free xT/wqT SBUF

        # ---------------- attention per head ----------------
        onorm = [None, None]
        oaccs = [None, None]

        def emit_norm(h, rs_pool, rs_tag):
            # partition-reduce rsacc via one ones-matmul (row sums replicated
            # across partitions), fast reciprocal, fold into O
            rs_ps = rs_pool.tile([128, T], f32, tag=rs_tag, name=f"rs_{h}")
            for c in range(2):
                cs = slice(c * 512, (c + 1) * 512)
                nc.tensor.matmul(rs_ps[:, cs], ones_sb[:], rsacc[h][:, cs],
                                 start=True, stop=True)
            rsinv = qtmp.tile([128, T], f32, tag="rsinv")
            nc.vector.reciprocal_approx_fast(out=rsinv[:], in_=rs_ps[:])
            oh = opool.tile([128, T], f16, tag="onorm", name=f"onorm{h}")
            for c in range(2):
                cs = slice(c * 512, (c + 1) * 512)
                nc.vector.tensor_mul(oh[:, cs], oaccs[h][:, cs], rsinv[:, cs])
            onorm[h] = oh

        for h in range(HLOC):
            oacc = (ps_acc if h == 0 else ps_q).tile(
                [128, T], f32, tag="acc" if h == 0 else "q", name=f"oacc{h}")
            oaccs[h] = oacc
            started = [False, False]
            for i in range(MT):
                if h == 1 and i == 14:
                    # h0's normalize emitted mid-h1-loop: late enough that
                    # h1's widest rsacc adds are past (DVE has slack), early
                    # enough to be off the outproj critical path
                    emit_norm(0, ps_main, "mm")
                chunk_lo, lo, b = win[i]
                sps = ps_main.tile([128, T], f32, tag="mm", name=f"s_{h}_{i}")
                for c in range(2):
                    clo = chunk_lo[c]
                    if clo is None:
                        continue
                    nc.tensor.matmul(
                        sps[:, clo:512 * (c + 1)],
                        ktr_sb[h][:, i * 128:(i + 1) * 128],
                        qtr[h][:, clo:512 * (c + 1)],
                        start=True, stop=True,
                    )
                p = ppool.tile([128, T], f16, tag="p")
                nc.scalar.activation(p[:, lo:], sps[:, lo:],
                                     mybir.ActivationFunctionType.Exp, scale=SCALE)
                if b > lo:
                    nc.vector.scalar_tensor_tensor(
                        out=p[:, lo:b], in0=posr_sb[:, lo:b],
                        scalar=miota_sb[:, i:i + 1], in1=p[:, lo:b],
                        op0=mybir.AluOpType.is_ge, op1=mybir.AluOpType.mult,
                    )
                for c in range(2):
                    clo = chunk_lo[c]
                    if clo is None:
                        continue
                    nc.tensor.matmul(
                        oacc[:, clo:512 * (c + 1)],
                        v_sb[h][:, i, :],
                        p[:, clo:512 * (c + 1)],
                        start=not started[c], stop=(i == last_act[c]),
                    )
                    started[c] = True
                nc.vector.tensor_add(rsacc[h][:, lo:], rsacc[h][:, lo:], p[:, lo:])
        # h1's normalize: rs_ps in ps_acc (oacc h0 freed after its normalize)
        # so outproj's first ps_main jps isn't gated on the reciprocal
        emit_norm(1, ps_acc, "acc")

        # ---------------- output projection ----------------
        # jps rotates over 4 PSUM homes (ps_main x2, ps_acc, ps_q -- the
        # accumulators are free post-normalize) so PE runs 4 deep ahead of
        # the copies; each copy is split scalar/vector halves in parallel.
        outT_r = d_out.rearrange("(jo p) t -> p jo t", p=128)
        for j in range(KO):
            r4 = j % 4
            if r4 == 1:
                jps = ps_acc.tile([128, T], f32, tag="acc", name=f"jps{j}")
            elif r4 == 3:
                jps = ps_q.tile([128, T], f32, tag="q", name=f"jps{j}")
            else:
                jps = ps_main.tile([128, T], f32, tag="mm", name=f"jps{j}")
            for c in range(2):
                cs = slice(c * 512, (c + 1) * 512)
                for ho in range(HLOC):
                    nc.tensor.matmul(
                        jps[:, cs],
                        woT_sb[:, ho, j * 128:(j + 1) * 128],
                        onorm[ho][:, cs],
                        start=(ho == 0), stop=(ho == HLOC - 1),
                    )
            ost = ostage.tile([128, T], f16, tag="ost")
            nc.scalar.copy(ost[:, 0:512], jps[:, 0:512])
            nc.vector.tensor_copy(ost[:, 512:], jps[:, 512:])
            if j >= KO - 4:
                # drain the last tiles as parallel half-DMAs on two rings
                e0, e1 = ((nc.sync, nc.gpsimd), (nc.scalar, nc.sync),
                          (nc.gpsimd, nc.scalar), (nc.sync, nc.gpsimd))[j % 4]
                e0.dma_start(out=outT_r[:, j, 0:512], in_=ost[:, 0:512])
                e1.dma_start(out=outT_r[:, j, 512:], in_=ost[:, 512:])
            else:
                dma_eng = (nc.sync, nc.gpsimd, nc.scalar)[j % 3]
                dma_eng.dma_start(out=outT_r[:, j, :], in_=ost[:])

    nc.compile()
    return nc


def _prep(inputs):
    """Host-side prep shared by kernel() and test harnesses."""
    x = np.asarray(inputs["x"], dtype=np.float32)
    k_ctx = np.asarray(inputs["k_ctx"], dtype=np.float32)
    v_ctx = np.asarray(inputs["v_ctx"], dtype=np.float32)
    W_q = np.asarray(inputs["W_q"], dtype=np.float32)
    W_o = np.asarray(inputs["W_o"], dtype=np.float32)
    pos_np = np.asarray(inputs["positions"]).astype(np.int64)
    pctx_np = np.asarray(inputs["p_ctx"]).astype(np.int64)

    perm = np.argsort(pos_np, kind="stable")
    ps = pos_np[perm]
    xs_T = x[perm].T.astype(np.float16)                                  # [D, T]
    xT = np.ascontiguousarray(
        xs_T.reshape(KO, 128, 2, 512).transpose(1, 2, 0, 3))             # [128,2,KO,512]
    k_rope = _host_rope_k(k_ctx, pctx_np).astype(np.float16)
    cosq, sinq = _host_q_tables(ps)
    cosq = cosq.astype(np.float16)
    sinq = sinq.astype(np.float16)
    posr = ps.astype(np.float32).reshape(1, T)
    miota = (np.arange(MT)[None, :] * 128 + np.arange(128)[:, None]).astype(np.float32)
    a_list = [int(np.searchsorted(ps, 128 * i, side="left")) for i in range(MT)]
    b_list = [int(np.searchsorted(ps, 128 * i + 127, side="left")) for i in range(MT)]

    in_maps = []
    for c in range(NCORES):
        hs = slice(c * HLOC * DK, (c + 1) * HLOC * DK)
        heads = range(c * HLOC, (c + 1) * HLOC)
        wq = W_q[hs, :].T.reshape(KO, 128, HLOC * DK).astype(np.float16)
        wo = W_o[:, hs].T.reshape(HLOC, 128, D).astype(np.float16)
        vv = v_ctx.transpose(1, 0, 2)[c * HLOC:(c + 1) * HLOC].astype(np.float16)
        in_maps.append({
            "xT": xT,
            "wqT": np.ascontiguousarray(wq.transpose(1, 0, 2)),
            "ktr": np.ascontiguousarray(np.stack([k_rope[:, h, :].T for h in heads])),
            "v": np.ascontiguousarray(vv.reshape(HLOC, MT, 128, DK).transpose(0, 2, 1, 3)),
            "woT": np.ascontiguousarray(wo.transpose(1, 0, 2)),
            "cosq": cosq, "sinq": sinq, "posr": posr, "miota": miota,
            "ones": np.ones((128, 128), dtype=np.float16),
        })
    return perm, a_list, b_list, in_maps


def kernel(x, k_ctx, v_ctx, W_q, W_o, positions, p_ctx):
    from concourse.bass_utils import run_bass_kernel_spmd

    inputs = dict(x=x, k_ctx=k_ctx, v_ctx=v_ctx, W_q=W_q, W_o=W_o,
                  positions=positions, p_ctx=p_ctx)
    perm, a_list, b_list, in_maps = _prep(inputs)

    key = (tuple(a_list), tuple(b_list))
    if key not in _PROGRAM_CACHE:
        _PROGRAM_CACHE[key] = _build_program(a_list, b_list)
    nc = _PROGRAM_CACHE[key]

    r = run_bass_kernel_spmd(nc, in_maps, core_ids=list(range(NCORES)))

    acc = np.zeros((D, T), dtype=np.float64)
    for c in range(NCORES):
        acc += r.results[c]["outT"].astype(np.float64)
    out_sorted = acc.T.astype(np.float32)
    out = np.empty_like(out_sorted)
    out[perm] = out_sorted
    return out.astype(np.float32)


if __name__ == "__main__":
    import importlib.util
    spec = importlib.util.spec_from_file_location("reference", "/root/problem/reference.py")
    ref = importlib.util.module_from_spec(spec)
    spec.loader.exec_module(ref)
    inputs = {k: np.asarray(v) for k, v in ref.setup_inputs().items()}
    expected = np.asarray(ref.reference(**inputs))
    got = kernel(**inputs)
    err = np.abs(got - expected)
    print("absmax err:", err.max(), "rel:", err.max() / np.abs(expected).max())
